# revision 40
# baseline (speedup 1.0000x reference)
"""Causal depthwise conv1d (K=4) + SiLU on TRN2 — channel-major fp16 design.

Device kernel per core (R=2048 out rows, D=2048 channels):

    DMA in (fp16, channel-major strips with per-channel weights embedded) ->
    K=4 accumulating diagonal matmuls per (d-block, l-chunk) on the PE
    (stationary = diag(w_k) fp16, moving = shifted strip slice, fp32 PSUM) ->
    ACT Silu (PSUM -> SBUF fp16) -> DMA out (fp16, channel-major)

The host pre-transposes each shard to channel-major fp16 (with K-1 halo
cols and the 4 weight taps prepended to every channel row) and
un-transposes/upcasts the output on gather. rel err ~1e-3 << 2e-2 gate.

Hardware facts learned from NTFF traces (drove this design):
 - PE runs the conv at 1 col/cycle (215 ns per 512-col matmul, LDWEIGHTS
   fully hidden): 55 us steady-state is this algorithm's floor.
 - DVE/GpSimd elementwise offload of conv blocks is 8-25x slower than
   nominal rates -> everything stays on PE.
 - A DMA into a [128, *] tile costs ~28 ns/descriptor * 128 descriptors
   ~= 3.6 us of queue-serial time REGARDLESS of bytes; queue throughput
   = bytes-per-partition/28ns. Hence: partition-major group tensors
   (one descriptor spans several blocks per partition), block 0 split
   across two queues by partition halves, big tail group DMA deferred
   via program order on the scalar engine, outputs alternating between
   two queues.
"""

from contextlib import ExitStack

import numpy as np

import concourse.bass as bass
import concourse.mybir as mybir
import concourse.tile as tile
from concourse.masks import make_identity

F16 = mybir.dt.float16
F32 = mybir.dt.float32
SILU = mybir.ActivationFunctionType.Silu
MULT = mybir.AluOpType.mult

# Input groups: (first block, #blocks). Group 0 is block 0 (split by
# partition halves over two queues for the earliest possible start);
# the last group is deferred and lands while blocks 0-7 compute.
GROUPS = ((0, 1), (1, 2), (3, 3), (6, 2), (8, 4), (12, 4))


def build_conv_kernel(
    nc: bass.Bass,
    R: int,            # output rows (l) per core
    D: int,            # channels (multiple of 128)
    K: int = 4,
    L_CHUNK: int = 512,
    pc_bufs: int = 8,
    ot_bufs: int = 4,
):
    HALO = K - 1
    NB = D // 128            # d-blocks of 128 channels
    RS = R + HALO            # strip length (halo prepended)
    WC = 2 * K               # fp32 weights bitcast into 2K fp16 cols
    ROW = WC + RS + 1        # per-channel row: [w fp32 x4 | halo | x | pad]
    assert ROW % 2 == 0      # even so the fp32 bitcast view works
    NCH = R // L_CHUNK       # l-chunks per block
    assert R % L_CHUNK == 0 and D % 128 == 0

    g_d = [nc.dram_tensor(f"g{i}", [128, n * ROW], F16, kind="ExternalInput")
           for i, (_, n) in enumerate(GROUPS)]
    o_d = nc.dram_tensor("out", [D, R], F16, kind="ExternalOutput")

    with ExitStack() as ctx:
        tc = ctx.enter_context(tile.TileContext(nc))

        const_pool = ctx.enter_context(tc.tile_pool(name="const", bufs=1))
        xt_pool = ctx.enter_context(tc.tile_pool(name="xt", bufs=1))
        ot_pool = ctx.enter_context(tc.tile_pool(name="ot", bufs=ot_bufs))
        pc_pool = ctx.enter_context(tc.tile_pool(name="pc", bufs=pc_bufs,
                                                 space="PSUM"))

        # Input group tiles. Group 0 (block 0): two partition-half DMAs on
        # two queues (64 descriptors each). Groups 1-3 on sync. Group 4
        # (blocks 8-15) is triggered LATER (from the scalar engine's
        # stream inside the conv loop) so it doesn't steal early DMA
        # bandwidth from the critical blocks.
        g_tiles = []
        for i, (gb, gn) in enumerate(GROUPS):
            t = xt_pool.tile([128, gn * ROW], F16, name=f"g{i}")
            g_tiles.append(t)
        nc.sync.dma_start(g_tiles[0][0:43, :], g_d[0][0:43, :])
        nc.scalar.dma_start(g_tiles[0][43:86, :], g_d[0][43:86, :])
        nc.gpsimd.dma_start(g_tiles[0][86:128, :], g_d[0][86:128, :])
        # sync's queue is FIFO: the tail groups (blocks 8-15) are LAST,
        # so they can't steal DMA bandwidth from the critical early blocks
        for i in (1, 2, 3, 4, 5):
            nc.sync.dma_start(g_tiles[i], g_d[i][:, :])

        # block -> (group tile, column base of that block's row)
        src = {}
        for i, (gb, gn) in enumerate(GROUPS):
            for j in range(gn):
                src[gb + j] = (g_tiles[i], j * ROW)

        ident = const_pool.tile([128, 128], F32)
        make_identity(nc, ident)
        ident16 = const_pool.tile([128, 128], F16)
        nc.vector.tensor_copy(ident16, ident)

        # diag(w[:, b, k]) fp16, one broadcast DVE instr per block:
        # diags3[p, b*K+k, f] = ident[p, f] * w_b[p, k]
        diags = const_pool.tile([128, NB * K * 128], F16)
        diags3 = diags.rearrange("p (c f) -> p c f", c=NB * K)
        ibc = ident16.rearrange("p (c f) -> p c f", c=1).broadcast_to(
            [128, K, 128])

        def diag_tt(b):
            t, base = src[b]
            nc.vector.tensor_tensor(
                diags3[:, b * K:(b + 1) * K, :],
                ibc,
                t[:, base:base + WC].bitcast(F32).rearrange(
                    "p (c f) -> p c f", f=1).broadcast_to([128, K, 128]),
                MULT,
            )

        for b in range(NB):
            diag_tt(b)

        for b in range(NB):
            ot = ot_pool.tile([128, R], F16, tag="ot")
            t, base = src[b]
            for c in range(NCH):
                pc = pc_pool.tile([128, L_CHUNK], F32, tag="pc")
                for k in range(K):
                    nc.tensor.matmul(
                        pc,
                        diags[:, (b * K + k) * 128:(b * K + k + 1) * 128],
                        t[:, base + WC + c * L_CHUNK + k:
                          base + WC + c * L_CHUNK + k + L_CHUNK],
                        start=(k == 0),
                        stop=(k == K - 1),
                    )
                nc.scalar.activation(ot[:, c * L_CHUNK:(c + 1) * L_CHUNK],
                                     pc, SILU)
            if b == NB - 1:
                # split the last output by partition thirds across all
                # three (otherwise-idle) queues: shortest tail descriptor time
                nc.gpsimd.dma_start(o_d[b * 128:b * 128 + 43, :], ot[0:43, :])
                nc.scalar.dma_start(o_d[b * 128 + 43:b * 128 + 86, :],
                                    ot[43:86, :])
                nc.sync.dma_start(o_d[b * 128 + 86:(b + 1) * 128, :],
                                  ot[86:128, :])
            elif b % 2 == 0:
                nc.gpsimd.dma_start(o_d[b * 128:(b + 1) * 128, :], ot)
            else:
                nc.scalar.dma_start(o_d[b * 128:(b + 1) * 128, :], ot)

    return nc


# ---------------------------------------------------------------------------
# Entry point: full (unsharded) inputs -> full output, 8 NeuronCores.
# ---------------------------------------------------------------------------
from concourse.bass_utils import run_bass_kernel_spmd
import concourse.bacc as bacc

_B, _L, _D, _K = 4, 4096, 2048, 4
_N_CORES = 8
_SHARDS_PER_BATCH = _N_CORES // _B
_LC = _L // _SHARDS_PER_BATCH     # 2048 output rows per core
_HALO = _K - 1
_ROW = 2 * _K + _LC + _HALO + 1   # [w fp32 x4 (bitcast) | halo | x | pad]
_NB = _D // 128

TRACE = False
LAST_EXEC_TIME_NS = None

_compiled_nc = None


def _get_nc():
    global _compiled_nc
    if _compiled_nc is None:
        nc = bacc.Bacc("TRN2", target_bir_lowering=False, debug=False)
        build_conv_kernel(nc, _LC, _D, K=_K, L_CHUNK=512)
        nc.compile()
        _compiled_nc = nc
    return _compiled_nc


def kernel(inputs: np.ndarray, weight: np.ndarray) -> np.ndarray:
    """inputs: (4, 4096, 2048) fp32; weight: (2048, 1, 4) fp32.

    Returns silu(causal_depthwise_conv1d(inputs, weight)): (4, 4096, 2048).
    Sharding: data parallel over (batch, L-chunk); each core's shard is
    pre-transposed to channel-major fp16 (weights + halo prepended per
    channel row) and regrouped partition-major per DMA group host-side.
    """
    global LAST_EXEC_TIME_NS
    x_full = np.asarray(inputs, dtype=np.float32)
    w_full = np.asarray(weight, dtype=np.float32)
    assert x_full.shape == (_B, _L, _D), x_full.shape

    # fp32 weights bitcast into pairs of fp16 slots per channel row
    w32 = np.ascontiguousarray(w_full.reshape(_D, _K).astype(np.float32))
    w_as16 = w32.view(np.float16)                      # [d, 2K]

    in_maps = []
    for c in range(_N_CORES):
        b, s = divmod(c, _SHARDS_PER_BATCH)
        l0 = s * _LC
        strip = np.empty((_D, _ROW), dtype=np.float16)
        strip[:, :2 * _K] = w_as16
        if s == 0:
            strip[:, 2 * _K:2 * _K + _HALO] = 0.0
        else:
            strip[:, 2 * _K:2 * _K + _HALO] = x_full[b, l0 - _HALO:l0].T
        strip[:, 2 * _K + _HALO:-1] = x_full[b, l0:l0 + _LC].T
        strip[:, -1] = 0.0
        # partition-major group tensors: g[p, j*ROW:(j+1)*ROW] is the row
        # of channel (gb+j)*128 + p
        s3 = strip.reshape(_NB, 128, _ROW)
        im = {}
        for i, (gb, gn) in enumerate(GROUPS):
            im[f"g{i}"] = np.ascontiguousarray(
                s3[gb:gb + gn].transpose(1, 0, 2).reshape(128, gn * _ROW))
        in_maps.append(im)

    nc = _get_nc()
    res = run_bass_kernel_spmd(nc, in_maps, list(range(_N_CORES)),
                               trace=TRACE)
    LAST_EXEC_TIME_NS = res.exec_time_ns

    out = np.empty((_B, _L, _D), dtype=np.float32)
    for c in range(_N_CORES):
        b, s = divmod(c, _SHARDS_PER_BATCH)
        out[b, s * _LC:(s + 1) * _LC] = res.results[c]["out"].T.astype(
            np.float32)
    return out


# revision 41
# speedup vs baseline: 1.3786x; 1.3786x over previous
"""Causal depthwise conv1d (K=4) + SiLU on TRN2 — channel-major fp16 design.

Device kernel per core (R=2048 out rows, D=2048 channels):

    DMA in (fp16, channel-major strips with per-channel weights embedded) ->
    K=4 accumulating diagonal matmuls per (d-block, l-chunk) on the PE
    (stationary = diag(w_k) fp16, moving = shifted strip slice, fp32 PSUM) ->
    ACT Silu (PSUM -> SBUF fp16) -> DMA out (fp16, channel-major)

The host pre-transposes each shard to channel-major fp16 (with K-1 halo
cols and the 4 weight taps prepended to every channel row) and
un-transposes/upcasts the output on gather. rel err ~1e-3 << 2e-2 gate.

Hardware facts learned from NTFF traces (drove this design):
 - PE runs the conv at 1 col/cycle (215 ns per 512-col matmul, LDWEIGHTS
   fully hidden): 55 us steady-state is this algorithm's floor.
 - DVE/GpSimd elementwise offload of conv blocks is 8-25x slower than
   nominal rates -> everything stays on PE.
 - A DMA into a [128, *] tile costs ~28 ns/descriptor * 128 descriptors
   ~= 3.6 us of queue-serial time REGARDLESS of bytes; queue throughput
   = bytes-per-partition/28ns. Hence: partition-major group tensors
   (one descriptor spans several blocks per partition), block 0 split
   across two queues by partition halves, big tail group DMA deferred
   via program order on the scalar engine, outputs alternating between
   two queues.
"""

from contextlib import ExitStack

import numpy as np

import concourse.bass as bass
import concourse.mybir as mybir
import concourse.tile as tile
from concourse.masks import make_identity

F16 = mybir.dt.float16
F32 = mybir.dt.float32
SILU = mybir.ActivationFunctionType.Silu
MULT = mybir.AluOpType.mult

# Input groups: (first block, #blocks). Group 0 is block 0 (split by
# partition halves over two queues for the earliest possible start);
# the last group is deferred and lands while blocks 0-7 compute.
GROUPS = ((0, 1), (1, 2), (3, 3), (6, 2), (8, 4), (12, 4))


def build_conv_kernel(
    nc: bass.Bass,
    R: int,            # output rows (l) per core
    D: int,            # channels (multiple of 128)
    K: int = 4,
    L_CHUNK: int = 512,
    pc_bufs: int = 8,
    ot_bufs: int = 4,
):
    HALO = K - 1
    NB = D // 128            # d-blocks of 128 channels
    RS = R + HALO            # strip length (halo prepended)
    WC = 2 * K               # fp32 weights bitcast into 2K fp16 cols
    ROW = WC + RS + 1        # per-channel row: [w fp32 x4 | halo | x | pad]
    assert ROW % 2 == 0      # even so the fp32 bitcast view works
    NCH = R // L_CHUNK       # l-chunks per block
    assert R % L_CHUNK == 0 and D % 128 == 0

    g_d = [nc.dram_tensor(f"g{i}", [128, n * ROW], F16, kind="ExternalInput")
           for i, (_, n) in enumerate(GROUPS)]
    o_d = nc.dram_tensor("out", [D, R], F16, kind="ExternalOutput")

    with ExitStack() as ctx:
        tc = ctx.enter_context(tile.TileContext(nc))

        const_pool = ctx.enter_context(tc.tile_pool(name="const", bufs=1))
        xt_pool = ctx.enter_context(tc.tile_pool(name="xt", bufs=1))
        ot_pool = ctx.enter_context(tc.tile_pool(name="ot", bufs=ot_bufs))
        pc_pool = ctx.enter_context(tc.tile_pool(name="pc", bufs=pc_bufs,
                                                 space="PSUM"))

        # Input group tiles. Group 0 (block 0): two partition-half DMAs on
        # two queues (64 descriptors each). Groups 1-3 on sync. Group 4
        # (blocks 8-15) is triggered LATER (from the scalar engine's
        # stream inside the conv loop) so it doesn't steal early DMA
        # bandwidth from the critical blocks.
        g_tiles = []
        for i, (gb, gn) in enumerate(GROUPS):
            t = xt_pool.tile([128, gn * ROW], F16, name=f"g{i}")
            g_tiles.append(t)
        nc.sync.dma_start(g_tiles[0][0:64, :], g_d[0][0:64, :])
        nc.scalar.dma_start(g_tiles[0][64:128, :], g_d[0][64:128, :])
        # sync's queue is FIFO: the tail groups (blocks 8-15) are LAST,
        # so they can't steal DMA bandwidth from the critical early blocks
        for i in (1, 2, 3, 4, 5):
            nc.sync.dma_start(g_tiles[i], g_d[i][:, :])

        # block -> (group tile, column base of that block's row)
        src = {}
        for i, (gb, gn) in enumerate(GROUPS):
            for j in range(gn):
                src[gb + j] = (g_tiles[i], j * ROW)

        ident = const_pool.tile([128, 128], F32)
        make_identity(nc, ident)
        ident16 = const_pool.tile([128, 128], F16)
        nc.vector.tensor_copy(ident16, ident)

        # diag(w[:, b, k]) fp16, one broadcast DVE instr per block:
        # diags3[p, b*K+k, f] = ident[p, f] * w_b[p, k]
        diags = const_pool.tile([128, NB * K * 128], F16)
        diags3 = diags.rearrange("p (c f) -> p c f", c=NB * K)
        ibc = ident16.rearrange("p (c f) -> p c f", c=1).broadcast_to(
            [128, K, 128])

        def diag_tt(b):
            t, base = src[b]
            nc.vector.tensor_tensor(
                diags3[:, b * K:(b + 1) * K, :],
                ibc,
                t[:, base:base + WC].bitcast(F32).rearrange(
                    "p (c f) -> p c f", f=1).broadcast_to([128, K, 128]),
                MULT,
            )

        for b in range(NB):
            diag_tt(b)

        for b in range(NB):
            ot = ot_pool.tile([128, R], F16, tag="ot")
            t, base = src[b]
            for c in range(NCH):
                pc = pc_pool.tile([128, L_CHUNK], F32, tag="pc")
                for k in range(K):
                    nc.tensor.matmul(
                        pc,
                        diags[:, (b * K + k) * 128:(b * K + k + 1) * 128],
                        t[:, base + WC + c * L_CHUNK + k:
                          base + WC + c * L_CHUNK + k + L_CHUNK],
                        start=(k == 0),
                        stop=(k == K - 1),
                    )
                nc.scalar.activation(ot[:, c * L_CHUNK:(c + 1) * L_CHUNK],
                                     pc, SILU)
            if b == NB - 1:
                # split the last output by partition halves across both
                # (otherwise-idle) queues: halves the tail descriptor time
                nc.gpsimd.dma_start(o_d[b * 128:b * 128 + 64, :], ot[0:64, :])
                nc.scalar.dma_start(o_d[b * 128 + 64:(b + 1) * 128, :],
                                    ot[64:128, :])
            elif b % 2 == 0:
                nc.gpsimd.dma_start(o_d[b * 128:(b + 1) * 128, :], ot)
            else:
                nc.scalar.dma_start(o_d[b * 128:(b + 1) * 128, :], ot)

    return nc


# ---------------------------------------------------------------------------
# Entry point: full (unsharded) inputs -> full output, 8 NeuronCores.
# ---------------------------------------------------------------------------
from concourse.bass_utils import run_bass_kernel_spmd
import concourse.bacc as bacc

_B, _L, _D, _K = 4, 4096, 2048, 4
_N_CORES = 8
_SHARDS_PER_BATCH = _N_CORES // _B
_LC = _L // _SHARDS_PER_BATCH     # 2048 output rows per core
_HALO = _K - 1
_ROW = 2 * _K + _LC + _HALO + 1   # [w fp32 x4 (bitcast) | halo | x | pad]
_NB = _D // 128

TRACE = False
LAST_EXEC_TIME_NS = None

_compiled_nc = None


def _get_nc():
    global _compiled_nc
    if _compiled_nc is None:
        nc = bacc.Bacc("TRN2", target_bir_lowering=False, debug=False)
        build_conv_kernel(nc, _LC, _D, K=_K, L_CHUNK=512)
        nc.compile()
        _compiled_nc = nc
    return _compiled_nc


def kernel(inputs: np.ndarray, weight: np.ndarray) -> np.ndarray:
    """inputs: (4, 4096, 2048) fp32; weight: (2048, 1, 4) fp32.

    Returns silu(causal_depthwise_conv1d(inputs, weight)): (4, 4096, 2048).
    Sharding: data parallel over (batch, L-chunk); each core's shard is
    pre-transposed to channel-major fp16 (weights + halo prepended per
    channel row) and regrouped partition-major per DMA group host-side.
    """
    global LAST_EXEC_TIME_NS
    x_full = np.asarray(inputs, dtype=np.float32)
    w_full = np.asarray(weight, dtype=np.float32)
    assert x_full.shape == (_B, _L, _D), x_full.shape

    # fp32 weights bitcast into pairs of fp16 slots per channel row
    w32 = np.ascontiguousarray(w_full.reshape(_D, _K).astype(np.float32))
    w_as16 = w32.view(np.float16)                      # [d, 2K]

    in_maps = []
    for c in range(_N_CORES):
        b, s = divmod(c, _SHARDS_PER_BATCH)
        l0 = s * _LC
        strip = np.empty((_D, _ROW), dtype=np.float16)
        strip[:, :2 * _K] = w_as16
        if s == 0:
            strip[:, 2 * _K:2 * _K + _HALO] = 0.0
        else:
            strip[:, 2 * _K:2 * _K + _HALO] = x_full[b, l0 - _HALO:l0].T
        strip[:, 2 * _K + _HALO:-1] = x_full[b, l0:l0 + _LC].T
        strip[:, -1] = 0.0
        # partition-major group tensors: g[p, j*ROW:(j+1)*ROW] is the row
        # of channel (gb+j)*128 + p
        s3 = strip.reshape(_NB, 128, _ROW)
        im = {}
        for i, (gb, gn) in enumerate(GROUPS):
            im[f"g{i}"] = np.ascontiguousarray(
                s3[gb:gb + gn].transpose(1, 0, 2).reshape(128, gn * _ROW))
        in_maps.append(im)

    nc = _get_nc()
    res = run_bass_kernel_spmd(nc, in_maps, list(range(_N_CORES)),
                               trace=TRACE)
    LAST_EXEC_TIME_NS = res.exec_time_ns

    out = np.empty((_B, _L, _D), dtype=np.float32)
    for c in range(_N_CORES):
        b, s = divmod(c, _SHARDS_PER_BATCH)
        out[b, s * _LC:(s + 1) * _LC] = res.results[c]["out"].T.astype(
            np.float32)
    return out


# revision 44
# speedup vs baseline: 1.3896x; 1.0080x over previous
"""Causal depthwise conv1d (K=4) + SiLU on TRN2 — channel-major fp16 design.

Device kernel per core (R=2048 out rows, D=2048 channels):

    DMA in (fp16, channel-major strips with per-channel weights embedded) ->
    K=4 accumulating diagonal matmuls per (d-block, l-chunk) on the PE
    (stationary = diag(w_k) fp16, moving = shifted strip slice, fp32 PSUM) ->
    ACT Silu (PSUM -> SBUF fp16) -> DMA out (fp16, channel-major)

The host pre-transposes each shard to channel-major fp16 (with K-1 halo
cols and the 4 weight taps prepended to every channel row) and
un-transposes/upcasts the output on gather. rel err ~1e-3 << 2e-2 gate.

Hardware facts learned from NTFF traces (drove this design):
 - PE runs the conv at 1 col/cycle (215 ns per 512-col matmul, LDWEIGHTS
   fully hidden): 55 us steady-state is this algorithm's floor.
 - DVE/GpSimd elementwise offload of conv blocks is 8-25x slower than
   nominal rates -> everything stays on PE.
 - A DMA into a [128, *] tile costs ~28 ns/descriptor * 128 descriptors
   ~= 3.6 us of queue-serial time REGARDLESS of bytes; queue throughput
   = bytes-per-partition/28ns. Hence: partition-major group tensors
   (one descriptor spans several blocks per partition), block 0 split
   across two queues by partition halves, tail groups LAST in the sync
   queue's FIFO (so they cannot steal early bandwidth), outputs
   alternating between the gpsimd and scalar queues, last output split
   by partition halves. Queue FIFO order is the only reliable pacing
   tool -- the tile scheduler reorders engine streams, and 3-way
   partition splits / input DMAs on the gpsimd queue both regressed.
"""

from contextlib import ExitStack

import numpy as np

import concourse.bass as bass
import concourse.mybir as mybir
import concourse.tile as tile
from concourse.masks import make_identity

F16 = mybir.dt.float16
F32 = mybir.dt.float32
SILU = mybir.ActivationFunctionType.Silu
MULT = mybir.AluOpType.mult

# Input groups: (first block, #blocks). Group 0 is block 0 (split by
# partition halves over two queues for the earliest possible start);
# the last group is deferred and lands while blocks 0-7 compute.
GROUPS = ((0, 1), (1, 2), (3, 3), (6, 2), (8, 4), (12, 4))


def build_conv_kernel(
    nc: bass.Bass,
    R: int,            # output rows (l) per core
    D: int,            # channels (multiple of 128)
    K: int = 4,
    L_CHUNK: int = 512,
    pc_bufs: int = 8,
    ot_bufs: int = 4,
):
    HALO = K - 1
    NB = D // 128            # d-blocks of 128 channels
    RS = R + HALO            # strip length (halo prepended)
    WC = 2 * K               # fp32 weights bitcast into 2K fp16 cols
    ROW = WC + RS + 1        # per-channel row: [w fp32 x4 | halo | x | pad]
    assert ROW % 2 == 0      # even so the fp32 bitcast view works
    NCH = R // L_CHUNK       # l-chunks per block
    assert R % L_CHUNK == 0 and D % 128 == 0

    g_d = [nc.dram_tensor(f"g{i}", [128, n * ROW], F16, kind="ExternalInput")
           for i, (_, n) in enumerate(GROUPS)]
    o_d = nc.dram_tensor("out", [D, R], F16, kind="ExternalOutput")

    with ExitStack() as ctx:
        tc = ctx.enter_context(tile.TileContext(nc))

        const_pool = ctx.enter_context(tc.tile_pool(name="const", bufs=1))
        xt_pool = ctx.enter_context(tc.tile_pool(name="xt", bufs=1))
        ot_pool = ctx.enter_context(tc.tile_pool(name="ot", bufs=ot_bufs))
        pc_pool = ctx.enter_context(tc.tile_pool(name="pc", bufs=pc_bufs,
                                                 space="PSUM"))

        # Input group tiles. Group 0 (block 0): two partition-half DMAs on
        # two queues (64 descriptors each, earliest possible first conv).
        # Groups 1-5 in FIFO order on sync: ascending by need-time, tail
        # groups last so they can't steal early DMA bandwidth.
        g_tiles = []
        for i, (gb, gn) in enumerate(GROUPS):
            t = xt_pool.tile([128, gn * ROW], F16, name=f"g{i}")
            g_tiles.append(t)
        nc.sync.dma_start(g_tiles[0][0:64, :], g_d[0][0:64, :])
        nc.scalar.dma_start(g_tiles[0][64:128, :], g_d[0][64:128, :])
        # sync's queue is FIFO: the tail groups (blocks 8-15) are LAST,
        # so they can't steal DMA bandwidth from the critical early blocks
        for i in (1, 2, 3, 4, 5):
            nc.sync.dma_start(g_tiles[i], g_d[i][:, :])

        # block -> (group tile, column base of that block's row)
        src = {}
        for i, (gb, gn) in enumerate(GROUPS):
            for j in range(gn):
                src[gb + j] = (g_tiles[i], j * ROW)

        ident = const_pool.tile([128, 128], F32)
        make_identity(nc, ident)
        ident16 = const_pool.tile([128, 128], F16)
        nc.vector.tensor_copy(ident16, ident)

        # diag(w[:, b, k]) fp16, one broadcast DVE instr per block:
        # diags3[p, b*K+k, f] = ident[p, f] * w_b[p, k]
        diags = const_pool.tile([128, NB * K * 128], F16)
        diags3 = diags.rearrange("p (c f) -> p c f", c=NB * K)
        ibc = ident16.rearrange("p (c f) -> p c f", c=1).broadcast_to(
            [128, K, 128])

        def diag_tt(b):
            t, base = src[b]
            nc.vector.tensor_tensor(
                diags3[:, b * K:(b + 1) * K, :],
                ibc,
                t[:, base:base + WC].bitcast(F32).rearrange(
                    "p (c f) -> p c f", f=1).broadcast_to([128, K, 128]),
                MULT,
            )

        for b in range(NB):
            diag_tt(b)

        for b in range(NB):
            ot = ot_pool.tile([128, R], F16, tag="ot")
            t, base = src[b]
            last = b == NB - 1
            for c in range(NCH):
                pc = pc_pool.tile([128, L_CHUNK], F32, tag="pc")
                for k in range(K):
                    nc.tensor.matmul(
                        pc,
                        diags[:, (b * K + k) * 128:(b * K + k + 1) * 128],
                        t[:, base + WC + c * L_CHUNK + k:
                          base + WC + c * L_CHUNK + k + L_CHUNK],
                        start=(k == 0),
                        stop=(k == K - 1),
                    )
                nc.scalar.activation(ot[:, c * L_CHUNK:(c + 1) * L_CHUNK],
                                     pc, SILU)
                if last and c % 2 == 1:
                    # last block: ship each finished column-half right
                    # away, split by partition halves over both idle
                    # queues -- the first pair's descriptor time overlaps
                    # the final convs, shortening the tail
                    c0 = (c - 1) * L_CHUNK
                    c1 = (c + 1) * L_CHUNK
                    nc.gpsimd.dma_start(
                        o_d[b * 128:b * 128 + 64, c0:c1], ot[0:64, c0:c1])
                    nc.scalar.dma_start(
                        o_d[b * 128 + 64:(b + 1) * 128, c0:c1],
                        ot[64:128, c0:c1])
            if last:
                pass
            elif b % 2 == 0:
                nc.gpsimd.dma_start(o_d[b * 128:(b + 1) * 128, :], ot)
            else:
                nc.scalar.dma_start(o_d[b * 128:(b + 1) * 128, :], ot)

    return nc


# ---------------------------------------------------------------------------
# Entry point: full (unsharded) inputs -> full output, 8 NeuronCores.
# ---------------------------------------------------------------------------
from concourse.bass_utils import run_bass_kernel_spmd
import concourse.bacc as bacc

_B, _L, _D, _K = 4, 4096, 2048, 4
_N_CORES = 8
_SHARDS_PER_BATCH = _N_CORES // _B
_LC = _L // _SHARDS_PER_BATCH     # 2048 output rows per core
_HALO = _K - 1
_ROW = 2 * _K + _LC + _HALO + 1   # [w fp32 x4 (bitcast) | halo | x | pad]
_NB = _D // 128

TRACE = False
LAST_EXEC_TIME_NS = None

_compiled_nc = None


def _get_nc():
    global _compiled_nc
    if _compiled_nc is None:
        nc = bacc.Bacc("TRN2", target_bir_lowering=False, debug=False)
        build_conv_kernel(nc, _LC, _D, K=_K, L_CHUNK=512)
        nc.compile()
        _compiled_nc = nc
    return _compiled_nc


def kernel(inputs: np.ndarray, weight: np.ndarray) -> np.ndarray:
    """inputs: (4, 4096, 2048) fp32; weight: (2048, 1, 4) fp32.

    Returns silu(causal_depthwise_conv1d(inputs, weight)): (4, 4096, 2048).
    Sharding: data parallel over (batch, L-chunk); each core's shard is
    pre-transposed to channel-major fp16 (weights + halo prepended per
    channel row) and regrouped partition-major per DMA group host-side.
    """
    global LAST_EXEC_TIME_NS
    x_full = np.asarray(inputs, dtype=np.float32)
    w_full = np.asarray(weight, dtype=np.float32)
    assert x_full.shape == (_B, _L, _D), x_full.shape

    # fp32 weights bitcast into pairs of fp16 slots per channel row
    w32 = np.ascontiguousarray(w_full.reshape(_D, _K).astype(np.float32))
    w_as16 = w32.view(np.float16)                      # [d, 2K]

    in_maps = []
    for c in range(_N_CORES):
        b, s = divmod(c, _SHARDS_PER_BATCH)
        l0 = s * _LC
        strip = np.empty((_D, _ROW), dtype=np.float16)
        strip[:, :2 * _K] = w_as16
        if s == 0:
            strip[:, 2 * _K:2 * _K + _HALO] = 0.0
        else:
            strip[:, 2 * _K:2 * _K + _HALO] = x_full[b, l0 - _HALO:l0].T
        strip[:, 2 * _K + _HALO:-1] = x_full[b, l0:l0 + _LC].T
        strip[:, -1] = 0.0
        # partition-major group tensors: g[p, j*ROW:(j+1)*ROW] is the row
        # of channel (gb+j)*128 + p
        s3 = strip.reshape(_NB, 128, _ROW)
        im = {}
        for i, (gb, gn) in enumerate(GROUPS):
            im[f"g{i}"] = np.ascontiguousarray(
                s3[gb:gb + gn].transpose(1, 0, 2).reshape(128, gn * _ROW))
        in_maps.append(im)

    nc = _get_nc()
    res = run_bass_kernel_spmd(nc, in_maps, list(range(_N_CORES)),
                               trace=TRACE)
    LAST_EXEC_TIME_NS = res.exec_time_ns

    out = np.empty((_B, _L, _D), dtype=np.float32)
    for c in range(_N_CORES):
        b, s = divmod(c, _SHARDS_PER_BATCH)
        out[b, s * _LC:(s + 1) * _LC] = res.results[c]["out"].T.astype(
            np.float32)
    return out
